# revision 11
# baseline (speedup 1.0000x reference)
"""DiT block kernel for Trainium2, data-parallel over batch across 8 NeuronCores.

Sharding: one batch element per core (B=8, n_cores=8), zero collectives.

Per-core shapes: x (1024,1024) f32, c_dino (1,1024) f32, c_textT (1024,128)
bf16, mask_bias (128,1) f32, weights bf16 (host-cast).

Layout strategy:
  - residual x kept natural (tokens on partitions, C free) in fp32
  - LN stats via bn_stats (reduce over free dim); normalized x is
    PE-transposed; modulate's per-channel shift/scale become per-partition
    scalars fused into the transpose eviction (ACT Identity scale/bias APs)
  - q^T,k^T via out^T = W.T @ h^T (lhsT=W natural); v natural via lhsT=h^T
  - scores built transposed S^T (keys on partitions, queries free): exp is
    the PSUM eviction (scale=D^-0.5 fused), the softmax denominator comes
    from a ones-augmented 65th column of V in the AV matmul, and the CA key
    mask is the per-partition exp bias
  - MLP: fc1 transposed out (gelu-tanh on eviction), fc2 natural (+residual)
"""

import numpy as np
import ml_dtypes
from contextlib import ExitStack

import concourse.mybir as mybir
import concourse.tile as tile
from concourse import bacc
from concourse.masks import make_identity

F32 = mybir.dt.float32
BF16 = mybir.dt.bfloat16
AF = mybir.ActivationFunctionType
ALU = mybir.AluOpType

B, N, C, H, D, M, MLPD, P = 8, 1024, 1024, 16, 64, 128, 4096, 128
NT = N // P       # 8 token tiles
CT = C // P       # 8 channel tiles
MT = MLPD // P    # 32 mlp tiles
EPS = 1e-6
SCALE = D ** -0.5
N_CORES = 8
NCHK = N // 512   # 2 query chunks


def build_kernel(repeat=1, gelu_composite=False):
    nc = bacc.Bacc("TRN2", target_bir_lowering=False, debug=False)

    x_d = nc.dram_tensor("x", [N, C], F32, kind="ExternalInput").ap()
    cdino_d = nc.dram_tensor("c_dino", [C], F32, kind="ExternalInput").ap()
    scr_ada = nc.dram_tensor("scr_ada", [6 * C], F32, kind="Internal").ap()
    ctextT_d = nc.dram_tensor("c_textT", [C, M], BF16,
                              kind="ExternalInput").ap()
    maskb_d = nc.dram_tensor("mask_bias", [M, 1], F32,
                             kind="ExternalInput").ap()
    wada_d = nc.dram_tensor("Wada", [C, 6 * C], BF16, kind="ExternalInput").ap()
    wqkv_d = nc.dram_tensor("Wqkv", [C, 3 * C], BF16, kind="ExternalInput").ap()
    wpsa_d = nc.dram_tensor("Wpsa", [C, C], BF16, kind="ExternalInput").ap()
    wq_d = nc.dram_tensor("Wq", [C, C], BF16, kind="ExternalInput").ap()
    wkv_d = nc.dram_tensor("Wkv", [C, 2 * C], BF16, kind="ExternalInput").ap()
    wpca_d = nc.dram_tensor("Wpca", [C, C], BF16, kind="ExternalInput").ap()
    wfc1_d = nc.dram_tensor("Wfc1", [C, MLPD], BF16, kind="ExternalInput").ap()
    wfc2_d = nc.dram_tensor("Wfc2", [MLPD, C], BF16, kind="ExternalInput").ap()
    out_d = nc.dram_tensor("out", [N, C], F32, kind="ExternalOutput").ap()

    with tile.TileContext(nc) as tc:
      with ExitStack() as ctx:
        const_pool = ctx.enter_context(tc.tile_pool(name="const", bufs=1))
        x_pool = ctx.enter_context(tc.tile_pool(name="x", bufs=1))
        stats_pool = ctx.enter_context(tc.tile_pool(name="stats", bufs=4))
        pmm = ctx.enter_context(tc.tile_pool(name="pmm", bufs=3, space="PSUM"))
        ps_s = ctx.enter_context(tc.tile_pool(name="ps_s", bufs=2,
                                              space="PSUM"))
        ps_av = ctx.enter_context(tc.tile_pool(name="ps_av", bufs=2,
                                               space="PSUM"))

        identity = const_pool.tile([P, P], BF16, name="identity")
        make_identity(nc, identity[:])
        maskb = const_pool.tile([M, 1], F32, name="maskb")
        ctT = const_pool.tile([P, CT, M], BF16, name="ctT")
        # ada working tiles (live across the whole block)
        cd_cols = const_pool.tile([P, CT], F32, name="cd_cols")
        sig_cols = const_pool.tile([P, CT], F32, name="sig_cols")
        sil_cols = const_pool.tile([P, CT], BF16, name="sil_cols")
        ada_row = const_pool.tile([1, 6 * C], F32, name="ada_row")
        ada_cols = const_pool.tile([P, 48], F32, name="ada_cols")
        eps_col = const_pool.tile([P, 1], F32, name="eps_col")
        nc.any.memset(eps_col[:], EPS)

        x_sb = [x_pool.tile([P, C], F32, name=f"x_{tt}") for tt in range(NT)]

        def ln_mod_transpose(stage, xn_pool, hT):
            """LN(x) -> (transpose) -> modulate; fills hT (128, CT, N) bf16."""
            xn_tiles = []
            for tt in range(NT):
                stats = stats_pool.tile([P, 2, 6], F32, tag="st",
                                        name=f"st{stage}_{tt}")
                nc.vector.bn_stats(out=stats[:, 0, :], in_=x_sb[tt][:, 0:512])
                nc.vector.bn_stats(out=stats[:, 1, :],
                                   in_=x_sb[tt][:, 512:1024])
                mv = stats_pool.tile([P, 2], F32, tag="mv",
                                     name=f"mv{stage}_{tt}")
                nc.vector.bn_aggr(out=mv[:], in_=stats[:])
                std = stats_pool.tile([P, 1], F32, tag="sd",
                                      name=f"sd{stage}_{tt}")
                nc.scalar.activation(std[:], mv[:, 1:2], AF.Sqrt,
                                     bias=eps_col[:])
                rstd = stats_pool.tile([P, 1], F32, tag="rs",
                                       name=f"rs{stage}_{tt}")
                nc.vector.reciprocal(rstd[:], std[:])
                xn = xn_pool.tile([P, C], BF16, name=f"xn{stage}_{tt}")
                nc.vector.tensor_scalar(
                    out=xn[:], in0=x_sb[tt][:], scalar1=mv[:, 0:1],
                    scalar2=rstd[:], op0=ALU.subtract, op1=ALU.mult)
                xn_tiles.append(xn)
            for ct in range(CT):
                sh_col = ada_cols[:, stage * 16 + ct: stage * 16 + ct + 1]
                sc_col = ada_cols[:, stage * 16 + 8 + ct:
                                  stage * 16 + 8 + ct + 1]
                for tg in range(NT // 4):
                    pt = pmm.tile([P, 512], BF16, tag="pmm",
                                  name=f"ptr{stage}_{ct}_{tg}")
                    for j in range(4):
                        tt = tg * 4 + j
                        nc.tensor.transpose(
                            pt[:, j * 128:(j + 1) * 128],
                            xn_tiles[tt][:, ct * 128:(ct + 1) * 128],
                            identity[:])
                    nc.scalar.activation(
                        hT[:, ct, tg * 512:(tg + 1) * 512], pt[:],
                        AF.Identity, bias=sh_col, scale=sc_col)

        def mm_T_out(wpool, dram_w, hT, out_cb, m_tiles, cols_off, name):
            """out^T[m] = W[:, cols].T @ h^T, streaming W in 4-mtile chunks."""
            chunk_m = 4
            for mg in range((m_tiles + chunk_m - 1) // chunk_m):
                m0 = mg * chunk_m
                mw = min(chunk_m, m_tiles - m0) * 128
                wt = wpool.tile([P, CT, chunk_m * 128], BF16,
                                tag=f"wc_{name}", name=f"w_{name}_{mg}")
                for kt in range(CT):
                    nc.sync.dma_start(
                        out=wt[:, kt, 0:mw],
                        in_=dram_w[kt * 128:(kt + 1) * 128,
                                   cols_off + m0 * 128:
                                   cols_off + m0 * 128 + mw])
                for mloc in range(mw // 128):
                    m = m0 + mloc
                    for nchk in range(NCHK):
                        ps = pmm.tile([P, 512], F32, tag="pmm",
                                      name=f"ps_{name}_{m}_{nchk}")
                        for kt in range(CT):
                            nc.tensor.matmul(
                                ps[:],
                                wt[:, kt, mloc * 128:(mloc + 1) * 128],
                                hT[:, kt, nchk * 512:(nchk + 1) * 512],
                                start=(kt == 0), stop=(kt == CT - 1))
                        out_cb(m, nchk, ps)

        def emit():
            for tt in range(NT):
                nc.sync.dma_start(out=x_sb[tt][:],
                                  in_=x_d[tt * 128:(tt + 1) * 128, :])
            nc.sync.dma_start(out=maskb[:], in_=maskb_d)
            for ct in range(CT):
                nc.sync.dma_start(out=ctT[:, ct, :],
                                  in_=ctextT_d[ct * 128:(ct + 1) * 128, :])

            # ---------------- ada = silu(c_dino) @ W_ada ----------------
            nc.sync.dma_start(out=cd_cols[:],
                              in_=cdino_d.rearrange("(t p) -> p t", p=P))
            nc.scalar.activation(sig_cols[:], cd_cols[:], AF.Sigmoid)
            nc.vector.tensor_mul(sil_cols[:], cd_cols[:], sig_cols[:])
            with tc.tile_pool(name="wada", bufs=2) as wada_pool:
                for cg in range(3):
                    wt = wada_pool.tile([P, CT, 2048], BF16, tag="wada",
                                        name=f"wada{cg}")
                    for kt in range(CT):
                        nc.sync.dma_start(
                            out=wt[:, kt, :],
                            in_=wada_d[kt * 128:(kt + 1) * 128,
                                       cg * 2048:(cg + 1) * 2048])
                    for j in range(4):
                        nchnk = cg * 4 + j
                        ps = ps_s.tile([1, 512], F32, tag="ps_s",
                                       name=f"psada{nchnk}")
                        for kt in range(CT):
                            nc.tensor.matmul(
                                ps[:], sil_cols[:, kt:kt + 1],
                                wt[:, kt, j * 512:(j + 1) * 512],
                                start=(kt == 0), stop=(kt == CT - 1))
                        nc.vector.tensor_copy(
                            ada_row[:, nchnk * 512:(nchnk + 1) * 512], ps[:])
            nc.sync.dma_start(out=scr_ada.rearrange("(o c) -> o c", o=1),
                              in_=ada_row[:])
            nc.sync.dma_start(out=ada_cols[:],
                              in_=scr_ada.rearrange("(t p) -> p t", p=P))
            for s in range(3):
                blk = ada_cols[:, s * 16 + 8: s * 16 + 16]
                nc.vector.tensor_scalar_add(out=blk, in0=blk, scalar1=1.0)

            # ================= self-attention =================
            with tc.tile_pool(name="qT", bufs=1) as qT_pool, \
                 tc.tile_pool(name="kT", bufs=1) as kT_pool, \
                 tc.tile_pool(name="vau", bufs=1) as vau_pool, \
                 tc.tile_pool(name="attnT", bufs=1) as attnT_pool:
                qT = qT_pool.tile([P, CT, N], BF16, name="qT")
                kT = kT_pool.tile([P, CT, N], BF16, name="kT")
                vau = vau_pool.tile([P, NT, H, 65], BF16, name="vau")
                saT = attnT_pool.tile([P, CT, N], BF16, name="saT")
                nc.any.memset(vau[:, :, :, 64:65], 1.0)

                with tc.tile_pool(name="xn1", bufs=1) as xn_pool, \
                     tc.tile_pool(name="hT1", bufs=1) as hT_pool:
                    hT = hT_pool.tile([P, CT, N], BF16, name="hT1")
                    ln_mod_transpose(0, xn_pool, hT)

                    with tc.tile_pool(name="wqk", bufs=2) as wqk_pool, \
                         tc.tile_pool(name="wv", bufs=1) as wv_pool:
                        def qk_evict(m, nchk, ps):
                            dst = qT if m < CT else kT
                            nc.any.tensor_copy(
                                dst[:, m % CT, nchk * 512:(nchk + 1) * 512],
                                ps[:])
                        mm_T_out(wqk_pool, wqkv_d, hT, qk_evict, 2 * CT, 0,
                                 "qk")
                        wv = wv_pool.tile([P, CT, C], BF16, name="wv")
                        for kt in range(CT):
                            nc.sync.dma_start(
                                out=wv[:, kt, :],
                                in_=wqkv_d[kt * 128:(kt + 1) * 128,
                                           2 * C:3 * C])
                        for tt in range(NT):
                            for nchk in range(NCHK):
                                ps = pmm.tile([P, 512], F32, tag="pmm",
                                              name=f"psv{tt}_{nchk}")
                                for kt in range(CT):
                                    nc.tensor.matmul(
                                        ps[:],
                                        hT[:, kt, tt * 128:(tt + 1) * 128],
                                        wv[:, kt,
                                           nchk * 512:(nchk + 1) * 512],
                                        start=(kt == 0), stop=(kt == CT - 1))
                                nc.any.tensor_copy(
                                    vau[:, tt, 8 * nchk:8 * (nchk + 1), 0:64],
                                    ps[:])

                with tc.tile_pool(name="PT", bufs=10) as pt_pool, \
                     tc.tile_pool(name="rcp", bufs=4) as rcp_pool, \
                     tc.tile_pool(name="bcs", bufs=4) as bcs_pool:
                    for h in range(H):
                        pb = (h % 2) * 64
                        ct = h // 2
                        for qc in range(NCHK):
                            pts = []
                            for kt in range(NT):
                                ps = ps_s.tile([P, 512], F32, tag="ps_s",
                                               name=f"S{h}_{qc}_{kt}")
                                nc.tensor.matmul(
                                    ps[:],
                                    kT[pb:pb + 64, ct,
                                       kt * 128:(kt + 1) * 128],
                                    qT[pb:pb + 64, ct,
                                       qc * 512:(qc + 1) * 512],
                                    start=True, stop=True)
                                pt = pt_pool.tile([P, 512], BF16, tag="pt",
                                                  name=f"P{h}_{qc}_{kt}")
                                nc.scalar.activation(pt[:], ps[:], AF.Exp,
                                                     scale=SCALE)
                                pts.append(pt)
                            av = ps_av.tile([65, 512], F32, tag="ps_av",
                                            name=f"av{h}_{qc}")
                            for kt in range(NT):
                                nc.tensor.matmul(
                                    av[:], vau[:, kt, h, :], pts[kt][:],
                                    start=(kt == 0), stop=(kt == NT - 1))
                            rc = rcp_pool.tile([1, 512], F32, tag="rc",
                                               name=f"rc{h}_{qc}")
                            nc.vector.reciprocal(rc[:], av[64:65, :])
                            bc = bcs_pool.tile([64, 512], F32, tag="bc",
                                               name=f"bc{h}_{qc}")
                            nc.gpsimd.partition_broadcast(bc[:], rc[:])
                            nc.vector.tensor_mul(
                                saT[pb:pb + 64, ct, qc * 512:(qc + 1) * 512],
                                av[0:64, :], bc[:])

                with tc.tile_pool(name="wpsa", bufs=1) as wpsa_pool:
                    wp = wpsa_pool.tile([P, CT, C], BF16, name="wpsa")
                    for kt in range(CT):
                        nc.sync.dma_start(
                            out=wp[:, kt, :],
                            in_=wpsa_d[kt * 128:(kt + 1) * 128, :])
                    for tt in range(NT):
                        for nchk in range(NCHK):
                            ps = pmm.tile([P, 512], F32, tag="pmm",
                                          name=f"psp{tt}_{nchk}")
                            for kt in range(CT):
                                nc.tensor.matmul(
                                    ps[:],
                                    saT[:, kt, tt * 128:(tt + 1) * 128],
                                    wp[:, kt, nchk * 512:(nchk + 1) * 512],
                                    start=(kt == 0), stop=(kt == CT - 1))
                            sl = slice(nchk * 512, (nchk + 1) * 512)
                            nc.vector.tensor_add(x_sb[tt][:, sl],
                                                 x_sb[tt][:, sl], ps[:])

            # ================= cross-attention =================
            with tc.tile_pool(name="qcT", bufs=1) as qcT_pool, \
                 tc.tile_pool(name="kvca", bufs=1) as kvca_pool, \
                 tc.tile_pool(name="attnT2", bufs=1) as attnT2_pool:
                qcT = qcT_pool.tile([P, CT, N], BF16, name="qcT")
                kcT = kvca_pool.tile([P, CT, M], BF16, name="kcT")
                vca = kvca_pool.tile([P, H, 65], BF16, name="vca")
                caT = attnT2_pool.tile([P, CT, N], BF16, name="caT")
                nc.any.memset(vca[:, :, 64:65], 1.0)

                with tc.tile_pool(name="xn2", bufs=1) as xn_pool, \
                     tc.tile_pool(name="hT2", bufs=1) as hT_pool:
                    hT = hT_pool.tile([P, CT, N], BF16, name="hT2")
                    ln_mod_transpose(1, xn_pool, hT)
                    with tc.tile_pool(name="wq2", bufs=2) as wq_pool:
                        def q_evict(m, nchk, ps):
                            nc.any.tensor_copy(
                                qcT[:, m, nchk * 512:(nchk + 1) * 512], ps[:])
                        mm_T_out(wq_pool, wq_d, hT, q_evict, CT, 0, "q2")

                with tc.tile_pool(name="wkv", bufs=1) as wkv_pool:
                    wkv = wkv_pool.tile([P, CT, 2 * C], BF16, name="wkv")
                    for kt in range(CT):
                        nc.sync.dma_start(
                            out=wkv[:, kt, :],
                            in_=wkv_d[kt * 128:(kt + 1) * 128, :])
                    for m in range(CT):
                        ps = pmm.tile([P, M], F32, tag="pmm",
                                      name=f"pskc{m}")
                        for kt in range(CT):
                            nc.tensor.matmul(
                                ps[:], wkv[:, kt, m * 128:(m + 1) * 128],
                                ctT[:, kt, :],
                                start=(kt == 0), stop=(kt == CT - 1))
                        nc.any.tensor_copy(kcT[:, m, :], ps[:])
                    for nchk in range(NCHK):
                        ps = pmm.tile([P, 512], F32, tag="pmm",
                                      name=f"psvc{nchk}")
                        for kt in range(CT):
                            nc.tensor.matmul(
                                ps[:], ctT[:, kt, :],
                                wkv[:, kt, C + nchk * 512:
                                    C + (nchk + 1) * 512],
                                start=(kt == 0), stop=(kt == CT - 1))
                        nc.any.tensor_copy(
                            vca[:, 8 * nchk:8 * (nchk + 1), 0:64], ps[:])

                with tc.tile_pool(name="PT2", bufs=6) as pt_pool, \
                     tc.tile_pool(name="rcp2", bufs=4) as rcp_pool, \
                     tc.tile_pool(name="bcs2", bufs=4) as bcs_pool:
                    for h in range(H):
                        pb = (h % 2) * 64
                        ct = h // 2
                        for qc in range(NCHK):
                            ps = ps_s.tile([P, 512], F32, tag="ps_s",
                                           name=f"Sc{h}_{qc}")
                            nc.tensor.matmul(
                                ps[:], kcT[pb:pb + 64, ct, :],
                                qcT[pb:pb + 64, ct,
                                    qc * 512:(qc + 1) * 512],
                                start=True, stop=True)
                            pt = pt_pool.tile([P, 512], BF16, tag="pt2",
                                              name=f"Pc{h}_{qc}")
                            nc.scalar.activation(pt[:], ps[:], AF.Exp,
                                                 scale=SCALE, bias=maskb[:])
                            av = ps_av.tile([65, 512], F32, tag="ps_av",
                                            name=f"avc{h}_{qc}")
                            nc.tensor.matmul(av[:], vca[:, h, :], pt[:],
                                             start=True, stop=True)
                            rc = rcp_pool.tile([1, 512], F32, tag="rc2",
                                               name=f"rcc{h}_{qc}")
                            nc.vector.reciprocal(rc[:], av[64:65, :])
                            bc = bcs_pool.tile([64, 512], F32, tag="bc2",
                                               name=f"bcc{h}_{qc}")
                            nc.gpsimd.partition_broadcast(bc[:], rc[:])
                            nc.vector.tensor_mul(
                                caT[pb:pb + 64, ct, qc * 512:(qc + 1) * 512],
                                av[0:64, :], bc[:])

                with tc.tile_pool(name="wpca", bufs=1) as wpca_pool:
                    wp = wpca_pool.tile([P, CT, C], BF16, name="wpca")
                    for kt in range(CT):
                        nc.sync.dma_start(
                            out=wp[:, kt, :],
                            in_=wpca_d[kt * 128:(kt + 1) * 128, :])
                    for tt in range(NT):
                        for nchk in range(NCHK):
                            ps = pmm.tile([P, 512], F32, tag="pmm",
                                          name=f"psq{tt}_{nchk}")
                            for kt in range(CT):
                                nc.tensor.matmul(
                                    ps[:],
                                    caT[:, kt, tt * 128:(tt + 1) * 128],
                                    wp[:, kt, nchk * 512:(nchk + 1) * 512],
                                    start=(kt == 0), stop=(kt == CT - 1))
                            sl = slice(nchk * 512, (nchk + 1) * 512)
                            nc.vector.tensor_add(x_sb[tt][:, sl],
                                                 x_sb[tt][:, sl], ps[:])

            # ================= MLP =================
            with tc.tile_pool(name="gT", bufs=1) as gT_pool:
                gT = gT_pool.tile([P, MT, N], BF16, name="gT")
                with tc.tile_pool(name="xn3", bufs=1) as xn_pool, \
                     tc.tile_pool(name="hT3", bufs=1) as hT_pool:
                    hT = hT_pool.tile([P, CT, N], BF16, name="hT3")
                    ln_mod_transpose(2, xn_pool, hT)
                    with tc.tile_pool(name="wfc1", bufs=2) as wfc1_pool, \
                         tc.tile_pool(name="gtmp", bufs=2) as gtmp_pool:
                        def g_evict(m, nchk, ps):
                            dst = gT[:, m, nchk * 512:(nchk + 1) * 512]
                            if not gelu_composite:
                                nc.scalar.activation(dst, ps[:],
                                                     AF.Gelu_apprx_tanh)
                                return
                            # 0.5x(1+tanh(0.79788456(x+0.044715x^3))), sim only
                            sq = gtmp_pool.tile([P, 512], F32, tag="gsq",
                                                name=f"gsq{m}_{nchk}")
                            nc.scalar.activation(sq[:], ps[:], AF.Square)
                            x3 = gtmp_pool.tile([P, 512], F32, tag="gx3",
                                                name=f"gx3{m}_{nchk}")
                            nc.vector.tensor_mul(x3[:], sq[:], ps[:])
                            nc.vector.tensor_scalar(
                                out=x3[:], in0=x3[:], scalar1=0.044715,
                                scalar2=None, op0=ALU.mult)
                            nc.vector.tensor_add(x3[:], x3[:], ps[:])
                            th = gtmp_pool.tile([P, 512], F32, tag="gth",
                                                name=f"gth{m}_{nchk}")
                            nc.scalar.activation(th[:], x3[:], AF.Tanh,
                                                 scale=0.7978845608028654)
                            nc.vector.tensor_scalar(
                                out=th[:], in0=th[:], scalar1=0.5,
                                scalar2=0.5, op0=ALU.mult, op1=ALU.add)
                            nc.vector.tensor_mul(dst, th[:], ps[:])
                        mm_T_out(wfc1_pool, wfc1_d, hT, g_evict, MT, 0, "fc1")

                with tc.tile_pool(name="wfc2", bufs=2) as wfc2_pool:
                    for half in range(2):
                        wf = wfc2_pool.tile([P, MT, 512], BF16, tag="wfc2",
                                            name=f"wfc2_{half}")
                        for kt in range(MT):
                            nc.sync.dma_start(
                                out=wf[:, kt, :],
                                in_=wfc2_d[kt * 128:(kt + 1) * 128,
                                           half * 512:(half + 1) * 512])
                        for tt in range(NT):
                            ps = pmm.tile([P, 512], F32, tag="pmm",
                                          name=f"psf{half}_{tt}")
                            for kt in range(MT):
                                nc.tensor.matmul(
                                    ps[:], gT[:, kt, tt * 128:(tt + 1) * 128],
                                    wf[:, kt, :],
                                    start=(kt == 0), stop=(kt == MT - 1))
                            sl = slice(half * 512, (half + 1) * 512)
                            nc.vector.tensor_add(x_sb[tt][:, sl],
                                                 x_sb[tt][:, sl], ps[:])

            for tt in range(NT):
                nc.sync.dma_start(out=out_d[tt * 128:(tt + 1) * 128, :],
                                  in_=x_sb[tt][:])

        if repeat == 1:
            emit()
        else:
            with tc.For_i(0, repeat, 1):
                emit()

    nc.compile()
    return nc


def _prep_in_maps(inputs):
    bf = ml_dtypes.bfloat16
    x = np.asarray(inputs["x"], np.float32)
    c_dino = np.asarray(inputs["c_dino"], np.float32)
    c_text = np.asarray(inputs["c_text"], np.float32)
    mask = np.asarray(inputs["text_mask"])
    w = {k: np.ascontiguousarray(np.asarray(inputs[k], np.float32).astype(bf))
         for k in ["W_ada", "W_qkv", "W_proj_sa", "W_q", "W_kv", "W_proj_ca",
                   "W_fc1", "W_fc2"]}
    in_maps = []
    for i in range(N_CORES):
        in_maps.append({
            "x": np.ascontiguousarray(x[i]),
            "c_dino": np.ascontiguousarray(c_dino[i]),
            "c_textT": np.ascontiguousarray(c_text[i].T.astype(bf)),
            "mask_bias": np.ascontiguousarray(
                ((mask[i].astype(np.float32) - 1.0) * 30000.0).reshape(M, 1)),
            "Wada": w["W_ada"], "Wqkv": w["W_qkv"], "Wpsa": w["W_proj_sa"],
            "Wq": w["W_q"], "Wkv": w["W_kv"], "Wpca": w["W_proj_ca"],
            "Wfc1": w["W_fc1"], "Wfc2": w["W_fc2"],
        })
    return in_maps


_NC_CACHE = {}


def get_nc(repeat=1):
    if repeat not in _NC_CACHE:
        _NC_CACHE[repeat] = build_kernel(repeat=repeat)
    return _NC_CACHE[repeat]


def kernel(**inputs):
    for bn in ["b_ada", "b_qkv", "b_proj_sa", "b_q", "b_kv", "b_proj_ca",
               "b_fc1", "b_fc2"]:
        if bn in inputs:
            assert float(np.abs(np.asarray(inputs[bn])).max()) == 0.0, \
                f"nonzero bias {bn} not supported"
    from concourse.bass_utils import run_bass_kernel_spmd
    nc = get_nc(repeat=1)
    in_maps = _prep_in_maps(inputs)
    res = run_bass_kernel_spmd(nc, in_maps, core_ids=list(range(N_CORES)))
    out = np.stack([res.results[i]["out"] for i in range(N_CORES)], axis=0)
    return out.astype(np.float32)
